# revision 44
# baseline (speedup 1.0000x reference)
"""Trainium2 Bass kernel for EnhancedGatedFusion (MoE routing, top-2 of 8).

Strategy: data-parallel over tokens across 8 NeuronCores, exploiting top-2
sparsity. The host computes the routing *permutation* (which expert each
token goes to) from its own logits and builds, per core, a compact
expert-sorted dispatch buffer xdT (bf16) plus gather-index tensors and a
per-slot rank sign (+1 for the token's top-1 slot, -1 for top-2). All
model arithmetic runs on device:

  Expert phase: for each expert block, token-major matmuls over its
     assigned token slots (bf16 at full PE rate). The router runs on the
     same resident dispatch tiles: logits -> max/2nd-max -> gate =
     sigmoid((m1-m2)*sign) per slot, so no separate router pass or gate
     gather is needed. SiLU (+bias when nonzero) is scaled by the gate and
     written to a dispatch-ordered HBM buffer yd.
  Combine phase: per 128-token tile, two indexed dma_gathers (transpose
     mode) pull each token's two gated expert outputs back into x^T
     layout; one DVE add combines them; then projection, residual add and
     RMSNorm.

Per-core matmul work drops from 8192 expert-slots (dense) to ~2688
(capacity-padded top-2). Zero biases / unit norm weights (as produced by
setup_inputs) compile to a specialized variant that skips the dead adds.
"""

import sys

for _p in ("/opt/trn_rl_repo",):
    if _p not in sys.path:
        sys.path.insert(0, _p)

from contextlib import ExitStack

import numpy as np

import concourse.bass as bass
import concourse.mybir as mybir
import concourse.tile as tile
from concourse import bacc

FP32 = mybir.dt.float32
BF16 = mybir.dt.bfloat16
I16 = mybir.dt.int16
AX = mybir.AxisListType
ALU = mybir.AluOpType
ACTF = mybir.ActivationFunctionType

EPS = 1e-6
NEG_BIG = -1e30


def _bcast_ap(ap, nparts=128):
    """Partition-broadcast view of a DRAM AP (step-0 partition dim)."""
    return bass.AP(tensor=ap.tensor, offset=ap.offset, ap=[[0, nparts], *ap.ap])


def build_moe_sparse(D, E, T, caps, flags, esplit, trn_type="TRN2"):
    """Per-core sparse-MoE program. caps[b] = token-slot capacity of the
    b-th processed expert block (multiple of 128, same on all cores; the
    host supplies weights in processing order). flags = (has_rb, has_eb,
    has_pb, has_nw) enable the bias/norm paths. esplit = number of trailing
    token tiles whose tokens may use the LAST expert block; earlier tiles
    gather from a sliced view of yd that excludes it, so their combines can
    start while the last expert is still running."""
    has_rb, has_eb, has_pb, has_nw = flags
    P = 128
    KO = D // P           # contraction k-tiles
    NTT = T // P          # token tiles
    NW = 512              # moving width (weight cols) per matmul
    NCP = D // NW         # col panels
    caps = list(caps)
    offs = np.concatenate([[0], np.cumsum(caps)]).astype(int)
    CAP = int(offs[-1])
    CMAX = max(caps)
    NSLT = CAP // P       # total slot tiles

    nc = bacc.Bacc(trn_type, target_bir_lowering=False, debug=False)

    xr = nc.dram_tensor("xr", [T, D], FP32, kind="ExternalInput").ap()
    # dispatch tokens pre-packed per block in SBUF-tile layout
    # [128, KO, cap_b] so each partition's block data is one contiguous run
    xds = nc.dram_tensor("xds", [P, KO * CAP], BF16, kind="ExternalInput").ap()
    idxA = nc.dram_tensor("idxA", [P, T // 16], I16, kind="ExternalInput").ap()
    idxB = nc.dram_tensor("idxB", [P, T // 16], I16, kind="ExternalInput").ap()
    sgn = nc.dram_tensor("sgn", [P, NSLT], FP32, kind="ExternalInput").ap()
    router_wb = nc.dram_tensor("router_wb", [D, E], BF16,
                               kind="ExternalInput").ap()
    router_b = nc.dram_tensor("router_b", [E], FP32, kind="ExternalInput").ap()
    # expert / proj weights pre-packed per 512-col panel as [128, KO*NW]
    expert_w = nc.dram_tensor("expert_w", [E, NCP, P, KO * NW], BF16,
                              kind="ExternalInput").ap()
    expert_b = nc.dram_tensor("expert_b", [E, D], FP32,
                              kind="ExternalInput").ap()
    proj_w = nc.dram_tensor("proj_w", [NCP, P, KO * NW], BF16,
                            kind="ExternalInput").ap()
    proj_b = nc.dram_tensor("proj_b", [D], FP32, kind="ExternalInput").ap()
    norm_w = nc.dram_tensor("norm_w", [D], FP32, kind="ExternalInput").ap()
    out = nc.dram_tensor("out", [T, D], FP32, kind="ExternalOutput").ap()
    yd = nc.dram_tensor("yd", [CAP, D], BF16).ap()

    rw_r = router_wb.rearrange("(ko p) e -> p ko e", p=P)

    with tile.TileContext(nc) as tc, ExitStack() as ctx:
        v = nc.vector
        s = nc.scalar

        singles = ctx.enter_context(tc.tile_pool(name="singles", bufs=1))

        # ---- residents ----
        rw_sb = singles.tile([P, KO, E], BF16)
        nc.sync.dma_start(out=rw_sb, in_=rw_r)
        idxA_sb = singles.tile([P, T // 16], I16)
        nc.sync.dma_start(out=idxA_sb, in_=idxA)
        idxB_sb = singles.tile([P, T // 16], I16)
        nc.sync.dma_start(out=idxB_sb, in_=idxB)
        sgn_sb = singles.tile([P, NSLT], FP32)
        nc.sync.dma_start(out=sgn_sb, in_=sgn)
        eps_t = singles.tile([P, 1], FP32)
        v.memset(eps_t, EPS)
        if has_rb:
            rb_rep = singles.tile([P, E], FP32)
            nc.sync.dma_start(out=rb_rep, in_=_bcast_ap(router_b))
        if has_pb:
            prb_rep = singles.tile([P, D], FP32)
            nc.sync.dma_start(out=prb_rep, in_=_bcast_ap(proj_b))
        if has_nw:
            nw_rep = singles.tile([P, D], FP32)
            nc.sync.dma_start(out=nw_rep, in_=_bcast_ap(norm_w))
        # proj weights resident (bf16, 8 MiB); DMAs issued at the end of the
        # expert phase (see below) so startup bandwidth goes to xde/wp
        pw_sb = singles.tile([P, KO, D], BF16)

        # ---- expert phase (router + experts fused, token-major) ----
        with (
            tc.tile_pool(name="xde", bufs=2) as xde_pool,
            tc.tile_pool(name="w_pool", bufs=4) as w_pool,
            tc.tile_pool(name="ebr", bufs=2) as ebr_pool,
            tc.tile_pool(name="gts", bufs=2) as gts_pool,
            tc.tile_pool(name="rch", bufs=3) as rch,
            tc.tile_pool(name="sil", bufs=4) as sil_pool,
            tc.tile_pool(name="pse", bufs=6, space="PSUM") as pse,
            tc.tile_pool(name="psg", bufs=2, space="PSUM") as psg,
        ):
            for e in range(E):
                ntiles = caps[e] // P
                base = int(offs[e])
                xde = xde_pool.tile([P, KO, CMAX], BF16, tag="xde",
                                    name=f"xde{e}")
                if e == 0:
                    # startup-critical: split across both queues and in ko
                    # quarters so the first router matmuls start early
                    KQ = KO // 4
                    for h in range(4):
                        heng = nc.sync if h % 2 == 0 else nc.scalar
                        c0 = KO * base + h * KQ * caps[e]
                        heng.dma_start(
                            out=xde[:, h * KQ:(h + 1) * KQ, :caps[e]],
                            in_=xds[:, c0:c0 + KQ * caps[e]].rearrange(
                                "p (ko t) -> p ko t", ko=KQ),
                        )
                else:
                    nc.scalar.dma_start(
                        out=xde[:, :, :caps[e]],
                        in_=xds[:, KO * base:KO * (base + caps[e])].rearrange(
                            "p (ko t) -> p ko t", ko=KO),
                    )
                if has_eb:
                    eb_rep = ebr_pool.tile([P, D], FP32, tag="ebr",
                                           name=f"ebr{e}")
                    nc.sync.dma_start(out=eb_rep, in_=_bcast_ap(expert_b[e]))
                # per-slot gates for this block: sigmoid((m1 - m2) * sign)
                gates = gts_pool.tile([P, ntiles], FP32, tag="gts",
                                      name=f"gt{e}")
                for ttl in range(ntiles):
                    ps_g = psg.tile([P, E], FP32, tag="psg",
                                    name=f"psg{e}_{ttl}")
                    for ko in range(KO):
                        nc.tensor.matmul(
                            ps_g,
                            lhsT=xde[:, ko, ttl * P:(ttl + 1) * P],
                            rhs=rw_sb[:, ko, :],
                            start=(ko == 0),
                            stop=(ko == KO - 1),
                        )
                    if has_rb:
                        logits = rch.tile([P, E], FP32, tag="lg",
                                          name=f"lg{e}_{ttl}")
                        v.tensor_tensor(out=logits, in0=ps_g, in1=rb_rep,
                                        op=ALU.add)
                    else:
                        logits = ps_g
                    m1 = rch.tile([P, 1], FP32, tag="m1", name=f"m1{e}_{ttl}")
                    v.tensor_reduce(m1, logits, axis=AX.X, op=ALU.max)
                    mask1 = rch.tile([P, E], FP32, tag="mk",
                                     name=f"mk{e}_{ttl}")
                    v.tensor_scalar(mask1, logits, m1, None, op0=ALU.is_ge)
                    lg2 = rch.tile([P, E], FP32, tag="lg2",
                                   name=f"lg2{e}_{ttl}")
                    v.scalar_tensor_tensor(
                        out=lg2, in0=mask1, scalar=NEG_BIG, in1=logits,
                        op0=ALU.mult, op1=ALU.add,
                    )
                    m2 = rch.tile([P, 1], FP32, tag="m2", name=f"m2{e}_{ttl}")
                    v.tensor_reduce(m2, lg2, axis=AX.X, op=ALU.max)
                    d12 = rch.tile([P, 1], FP32, tag="d12",
                                   name=f"d12{e}_{ttl}")
                    v.tensor_tensor(out=d12, in0=m1, in1=m2, op=ALU.subtract)
                    ds = rch.tile([P, 1], FP32, tag="ds", name=f"ds{e}_{ttl}")
                    gti = base // P + ttl
                    v.tensor_tensor(out=ds, in0=d12,
                                    in1=sgn_sb[:, gti:gti + 1], op=ALU.mult)
                    s.activation(gates[:, ttl:ttl + 1], ds, ACTF.Sigmoid)

                for cq in range(NCP):
                    wp = w_pool.tile([P, KO, NW], BF16, tag="wp",
                                     name=f"wp{e}_{cq}")
                    weng = nc.sync if cq % 2 == 0 else nc.scalar
                    weng.dma_start(
                        out=wp,
                        in_=expert_w[e, cq].rearrange(
                            "p (ko w) -> p ko w", ko=KO))
                    ysb = sil_pool.tile([P, ntiles, NW], BF16, tag="ysb",
                                        name=f"y{e}_{cq}")
                    for ttl in range(ntiles):
                        ps = pse.tile([P, NW], FP32, tag="ps",
                                      name=f"ps{e}_{cq}_{ttl}")
                        for ko in range(KO):
                            nc.tensor.matmul(
                                ps,
                                lhsT=xde[:, ko, ttl * P:(ttl + 1) * P],
                                rhs=wp[:, ko, :],
                                start=(ko == 0),
                                stop=(ko == KO - 1),
                            )
                        if has_eb:
                            hsb = sil_pool.tile([P, NW], FP32, tag="hsb",
                                                name=f"h{e}_{cq}_{ttl}")
                            v.tensor_tensor(
                                out=hsb, in0=ps,
                                in1=eb_rep[:, cq * NW:(cq + 1) * NW],
                                op=ALU.add)
                        else:
                            hsb = ps
                        s.activation(ysb[:, ttl, :], hsb, ACTF.Silu)
                        v.tensor_scalar(ysb[:, ttl, :], ysb[:, ttl, :],
                                        gates[:, ttl:ttl + 1],
                                        None, op0=ALU.mult)
                    # one batched store per (expert, panel): fewer DMA
                    # issues on the sync/scalar engine queues
                    oeng = nc.sync if cq % 2 == 0 else nc.scalar
                    oeng.dma_start(
                        out=yd[base:base + ntiles * P,
                               cq * NW:(cq + 1) * NW].rearrange(
                            "(t p) w -> p t w", p=P),
                        in_=ysb,
                    )

            # proj weights load behind the last expert's weights on the same
            # FIFO queues: they transfer while the PE crunches the last
            # expert and are resident before the combine phase needs them
            for pp in range(NCP):
                peng = nc.sync if pp % 2 == 0 else nc.scalar
                peng.dma_start(
                    out=pw_sb[:, :, pp * NW:(pp + 1) * NW],
                    in_=proj_w[pp].rearrange("p (ko w) -> p ko w", ko=KO))

        # ---- combine + projection + residual + RMSNorm ----
        with (
            tc.tile_pool(name="gpool", bufs=3) as g_pool,
            tc.tile_pool(name="ctp", bufs=3) as ct_pool,
            tc.tile_pool(name="ypool", bufs=3) as y_pool,
            tc.tile_pool(name="nsm", bufs=2) as nsm,
            tc.tile_pool(name="xres", bufs=3) as xres_pool,
            tc.tile_pool(name="psp", bufs=6, space="PSUM") as psp,
        ):
            def emit_norm(tt, y_t):
                HD = D // 2
                sq = nsm.tile([P, HD], FP32, tag="sq", bufs=1, name=f"sq{tt}")
                ssa = nsm.tile([P, 1], FP32, tag="ssa", name=f"ssa{tt}")
                ssb = nsm.tile([P, 1], FP32, tag="ssb", name=f"ssb{tt}")
                s.activation(sq, y_t[:, :HD], ACTF.Square, accum_out=ssa)
                s.activation(sq, y_t[:, HD:], ACTF.Square, accum_out=ssb)
                ssum = nsm.tile([P, 1], FP32, tag="ssum", name=f"ssum{tt}")
                v.tensor_tensor(out=ssum, in0=ssa, in1=ssb, op=ALU.add)
                rms = nsm.tile([P, 1], FP32, tag="rms", name=f"rms{tt}")
                s.activation(rms, ssum, ACTF.Sqrt, bias=eps_t, scale=1.0 / D)
                rinv = nsm.tile([P, 1], FP32, tag="rinv", name=f"rinv{tt}")
                v.reciprocal(rinv, rms)
                s.mul(y_t, y_t, rinv)
                if has_nw:
                    v.tensor_tensor(out=y_t, in0=y_t, in1=nw_rep, op=ALU.mult)
                oeng = nc.sync if tt % 2 == 0 else nc.scalar
                oeng.dma_start(out=out[tt * P:(tt + 1) * P, :], in_=y_t)

            for tt in range(NTT):
                # tokens in tiles [0, NTT-esplit) never reference the last
                # expert block, so their gathers read a sliced view of yd
                # that excludes it (lets them start before it finishes)
                ydv = yd if tt >= NTT - esplit else yd[:int(offs[E - 1])]
                gA = g_pool.tile([P, KO, P], BF16, tag="gA", name=f"gA{tt}")
                nc.gpsimd.dma_gather(
                    gA, ydv, idxA_sb[:, tt * 8:(tt + 1) * 8],
                    num_idxs=P, num_idxs_reg=P,
                    elem_size=D, transpose=True,
                )
                gB = g_pool.tile([P, KO, P], BF16, tag="gB", name=f"gB{tt}")
                nc.gpsimd.dma_gather(
                    gB, ydv, idxB_sb[:, tt * 8:(tt + 1) * 8],
                    num_idxs=P, num_idxs_reg=P,
                    elem_size=D, transpose=True,
                )
                ctt = ct_pool.tile([P, KO, P], BF16, tag="ct", name=f"ct{tt}")
                v.tensor_tensor(out=ctt, in0=gA, in1=gB, op=ALU.add)

                y_t = y_pool.tile([P, D], FP32, tag="yt", name=f"yt{tt}")
                # one residual load per tile (fewer DMA issues)
                xres = xres_pool.tile([P, D], FP32, tag="xres",
                                      name=f"xr{tt}")
                nc.scalar.dma_start(out=xres, in_=xr[tt * P:(tt + 1) * P, :])
                for pp in range(NCP):
                    ps_o = psp.tile([P, NW], FP32, tag="pso",
                                    name=f"pso{tt}_{pp}")
                    for ko in range(KO):
                        nc.tensor.matmul(
                            ps_o,
                            lhsT=ctt[:, ko, :],
                            rhs=pw_sb[:, ko, pp * NW:(pp + 1) * NW],
                            start=(ko == 0),
                            stop=(ko == KO - 1),
                        )
                    y_sl = y_t[:, pp * NW:(pp + 1) * NW]
                    xr_sl = xres[:, pp * NW:(pp + 1) * NW]
                    if has_pb:
                        v.tensor_tensor(
                            out=y_sl, in0=ps_o,
                            in1=prb_rep[:, pp * NW:(pp + 1) * NW], op=ALU.add)
                        v.tensor_tensor(out=y_sl, in0=y_sl, in1=xr_sl,
                                        op=ALU.add)
                    else:
                        v.tensor_tensor(out=y_sl, in0=ps_o, in1=xr_sl,
                                        op=ALU.add)
                emit_norm(tt, y_t)

    nc.compile()
    return nc


# ---- host-side routing / dispatch ----
_B, _S, _D, _E = 4, 2048, 2048, 8
_NCORES = 8
_T = _B * _S // _NCORES

_nc_cache = {}


def _get_nc(caps, flags, esplit):
    key = (tuple(caps), flags, esplit)
    if key not in _nc_cache:
        _nc_cache[key] = build_moe_sparse(_D, _E, _T, caps, flags, esplit)
    return _nc_cache[key]


def _route(xf, router_w, router_b):
    """Host routing decisions (dispatch permutation only - all model math
    is recomputed on device). Returns per-core token ids + top-2 experts."""
    logits = xf.astype(np.float64) @ router_w.astype(np.float64) \
        + router_b.astype(np.float64)
    order = np.argsort(-logits, axis=1, kind="stable")
    i1 = order[:, 0].astype(np.int64)
    i2 = order[:, 1].astype(np.int64)
    # deal tokens to cores round-robin in expert-pair order so every core
    # sees ~1/8 of each expert's tokens (balances per-expert capacities)
    deal = np.argsort(i1 * _E + i2, kind="stable")
    Tall = xf.shape[0]
    assign = np.empty(Tall, np.int64)
    assign[deal] = np.arange(Tall) % _NCORES
    tok_ids = [np.nonzero(assign == c)[0] for c in range(_NCORES)]
    return tok_ids, i1, i2


def _wrap_idx(idx):
    """[T] -> wrapped [128, T//16] int16 layout for dma_gather."""
    w16 = idx.reshape(-1, 16).T.astype(np.int16)   # [16, T//16]
    return np.ascontiguousarray(np.tile(w16, (8, 1)))


def _prepare(x, router_w, router_b, expert_w, expert_b, proj_w, proj_b,
             norm_w):
    import ml_dtypes
    BF = ml_dtypes.bfloat16

    xf = x.reshape(-1, _D)
    tok_ids, i1, i2 = _route(xf, router_w, router_b)

    counts = np.zeros((_NCORES, _E), np.int64)
    for c in range(_NCORES):
        ids = tok_ids[c]
        for e in range(_E):
            counts[c, e] = np.count_nonzero((i1[ids] == e) | (i2[ids] == e))
    caps_e = ((counts.max(axis=0) + 127) // 128) * 128
    # expert processing order: a small block first (fast start) and a small
    # block last (its completion gates the trailing token tiles)
    by_cap = sorted(range(_E), key=lambda e: (int(caps_e[e]), e))
    perm = [by_cap[0]] + sorted(by_cap[2:]) + [by_cap[1]]
    elast = perm[-1]
    caps = tuple(int(caps_e[e]) for e in perm)
    offs = np.concatenate([[0], np.cumsum(caps)]).astype(int)
    CAP = int(offs[-1])
    esplit = int((counts[:, elast].max() + 127) // 128)

    flags = (
        bool(np.any(router_b != 0.0)),
        bool(np.any(expert_b != 0.0)),
        bool(np.any(proj_b != 0.0)),
        bool(np.any(norm_w != 1.0)),
    )

    KO, NW = _D // 128, 512
    NCP = _D // NW
    # weights pre-packed per 512-col panel into SBUF-tile layout
    # [128, KO*NW] so every DMA descriptor is a 16 KB contiguous run
    ew_b = np.ascontiguousarray(
        expert_w[perm].astype(BF)
        .reshape(_E, KO, 128, NCP, NW)
        .transpose(0, 3, 2, 1, 4)
        .reshape(_E, NCP, 128, KO * NW))
    eb_p = np.ascontiguousarray(expert_b[perm])
    pw_b = np.ascontiguousarray(
        proj_w.astype(BF)
        .reshape(KO, 128, NCP, NW)
        .transpose(2, 1, 0, 3)
        .reshape(NCP, 128, KO * NW))
    rw_b = np.ascontiguousarray(router_w.astype(BF))

    in_maps = []
    new_tok_ids = []
    for c in range(_NCORES):
        ids = tok_ids[c]
        l1, l2 = i1[ids], i2[ids]
        # tokens that use the last-processed expert go to the trailing tiles
        uses_last = (l1 == elast) | (l2 == elast)
        ord2 = np.argsort(uses_last, kind="stable")
        ids = ids[ord2]
        l1, l2 = l1[ord2], l2[ord2]
        new_tok_ids.append(ids)
        xs = np.ascontiguousarray(xf[ids])           # [T, D] fp32
        slot_tok = np.zeros(CAP, np.int64)
        sgn_flat = np.zeros(CAP, np.float32)
        idxA = np.empty(_T, np.int64)
        idxB = np.empty(_T, np.int64)
        for b, e in enumerate(perm):
            tokA = np.nonzero(l1 == e)[0]
            tokB = np.nonzero(l2 == e)[0]
            base = int(offs[b])
            nA, nB = len(tokA), len(tokB)
            slot_tok[base:base + nA] = tokA
            slot_tok[base + nA:base + nA + nB] = tokB
            sgn_flat[base:base + nA] = 1.0
            sgn_flat[base + nA:base + nA + nB] = -1.0
            idxA[tokA] = base + np.arange(nA)
            idxB[tokB] = base + nA + np.arange(nB)
        xd = xs[slot_tok].astype(BF)                 # [CAP, D]
        xds = np.empty((128, KO * CAP), BF)
        for b in range(_E):
            base, cap = int(offs[b]), caps[b]
            blk = xd[base:base + cap].reshape(cap, KO, 128).transpose(2, 1, 0)
            xds[:, KO * base:KO * (base + cap)] = blk.reshape(128, KO * cap)
        m = {
            "xr": xs,
            "xds": xds,
            "idxA": _wrap_idx(idxA),
            "idxB": _wrap_idx(idxB),
            "sgn": np.ascontiguousarray(sgn_flat.reshape(-1, 128).T),
            "router_wb": rw_b,
            "router_b": router_b,
            "expert_w": ew_b,
            "expert_b": eb_p,
            "proj_w": pw_b,
            "proj_b": proj_b,
            "norm_w": norm_w,
        }
        in_maps.append(m)
    return caps, flags, esplit, in_maps, new_tok_ids


def kernel(x, router_w, router_b, expert_w, expert_b, proj_w, proj_b, norm_w):
    from concourse import bass_utils

    x = np.asarray(x, np.float32)
    router_w = np.asarray(router_w, np.float32)
    router_b = np.asarray(router_b, np.float32)
    expert_w = np.asarray(expert_w, np.float32)
    expert_b = np.asarray(expert_b, np.float32)
    proj_w = np.asarray(proj_w, np.float32)
    proj_b = np.asarray(proj_b, np.float32)
    norm_w = np.asarray(norm_w, np.float32)

    caps, flags, esplit, in_maps, tok_ids = _prepare(
        x, router_w, router_b, expert_w, expert_b, proj_w, proj_b, norm_w)
    nc = _get_nc(caps, flags, esplit)
    res = bass_utils.run_bass_kernel_spmd(
        nc, in_maps, core_ids=list(range(_NCORES)))
    full = np.empty((_B * _S, _D), np.float32)
    for c in range(_NCORES):
        full[tok_ids[c]] = res.results[c]["out"]
    return full.reshape(_B, _S, _D)
